# revision 8
# baseline (speedup 1.0000x reference)
"""Trainium2 Bass kernel for nn_CriticNetwork_71442486001992.

Pipeline (per full call):
  device (8 NeuronCores, SPMD, one NEFF):
    - conv3d(state)+bias+relu in two packed layouts (d-sharded and h-sharded
      across cores), via a block-diagonal weight matmul (K=48)
    - mode-1/2/3 Gram partials of the conv output (the heavy FLOPs of the
      tucker HOSVD init), partial-summed per core
  host (numpy / LAPACK):
    - gram assembly + mode-0 gram, eigh-based HOSVD init, 5 HOOI sweeps
      (tiny tensors after the first contraction), final MLP head
Sharding: cores split mode-1 (d: 96 -> 12/core) for G2/G3 and mode-2
(h: 384 -> 48/core) for G1; gram partials are summed on host (no
collectives needed).
"""

import numpy as np

# ---------------------------------------------------------------- constants
N_CORES = 8
TUCKER_RANKS = (8, 2, 6, 2)
N_ITER = 5

O, I, KH = 32, 4, 3           # conv out-ch, in-ch, taps (along h)
D, H, W = 96, 384, 24         # sv spatial dims (d, h, w); input h = 386
DL = D // N_CORES             # 12 d's per core (mode-1 shard)
HL = H // N_CORES             # 48 h's per core (mode-2 shard)
D4, H4 = DL // 4, HL // 4     # 3, 12  (4-way partition packing)
HB = 24                       # h chunked into 24 blocks of 16 for conv pack1
F1 = D4 * H * W               # 27648 free elems of sv1  (d4, h, w)
F2 = D * H4 * W               # 27648 free elems of sv2  (d, h4, w)

_CACHE = {}


# ---------------------------------------------------------------- device part
def _build_bass(num_devices=N_CORES):
    import concourse.mybir as mybir
    from concourse import bacc
    from concourse.tile import TileContext

    f32 = mybir.dt.float32
    nc = bacc.Bacc("TRN2", target_bir_lowering=False, debug=False,
                   num_devices=num_devices)

    # inputs (per core)
    st1 = nc.dram_tensor("st1", [48, F1], f32, kind="ExternalInput").ap()
    st2 = nc.dram_tensor("st2", [48, F2], f32, kind="ExternalInput").ap()
    w2bd = nc.dram_tensor("w2bd", [48, 128], f32, kind="ExternalInput").ap()
    bias4 = nc.dram_tensor("bias4", [128, 1], f32, kind="ExternalInput").ap()
    # outputs (per core)
    sv1_o = nc.dram_tensor("sv1", [128, F1], f32, kind="ExternalOutput").ap()
    g2_o = nc.dram_tensor("g2p", [3, 128, H], f32, kind="ExternalOutput").ap()
    g1_o = nc.dram_tensor("g1p", [D, D], f32, kind="ExternalOutput").ap()
    g3_o = nc.dram_tensor("g3p", [W, W], f32, kind="ExternalOutput").ap()

    with TileContext(nc) as tc:
        with tc.tile_pool(name="consts", bufs=1) as cpool:
            w2_t = cpool.tile([48, 128], f32)
            nc.sync.dma_start(out=w2_t[:], in_=w2bd[:])
            b_t = cpool.tile([128, 1], f32)
            nc.sync.dma_start(out=b_t[:], in_=bias4[:])

            # ---------------- phase 1: d-shard conv -> sv1; G2, G3 ----------
            with tc.tile_pool(name="p1", bufs=1) as p1, \
                 tc.tile_pool(name="p1in", bufs=3) as p1i, \
                 tc.tile_pool(name="ps1", bufs=2, space="PSUM") as ps1, \
                 tc.tile_pool(name="ev1", bufs=3) as ev1:
                sv1 = p1.tile([128, F1], f32)
                sv1r = sv1.rearrange("p (a h w) -> p a h w", a=D4, w=W)
                # conv: 6 input pieces of (d4, hb-half); chunk = 16 h rows
                for d4 in range(D4):
                    for half in range(2):
                        piece = p1i.tile([48, 12 * 384], f32, tag="st1piece")
                        off = d4 * (HB * 384) + half * (12 * 384)
                        nc.sync.dma_start(out=piece[:], in_=st1[:, off:off + 12 * 384])
                        for hb in range(12):
                            ps = ps1.tile([128, 384], f32, tag="convps")
                            nc.tensor.matmul(ps[:], w2_t[:], piece[:, hb * 384:(hb + 1) * 384],
                                             start=True, stop=True)
                            nc.scalar.activation(
                                sv1[:, off + hb * 384: off + (hb + 1) * 384], ps[:],
                                mybir.ActivationFunctionType.Relu, bias=b_t[:])
                # write sv1 back for the host
                nc.sync.dma_start(out=sv1_o[:], in_=sv1[:])
                # G2[h, h'] partial: contract (o, d, w); 3 psum row-blocks
                for mb in range(3):
                    acc = ps1.tile([128, H], f32, tag=f"g2acc{mb}", bufs=1)
                    n = 0
                    for d4 in range(D4):
                        for w in range(W):
                            nc.tensor.matmul(
                                acc[:],
                                sv1r[:, d4, mb * 128:(mb + 1) * 128, w],
                                sv1r[:, d4, :, w],
                                start=(n == 0), stop=(n == D4 * W - 1))
                            n += 1
                    ev = ev1.tile([128, H], f32, tag="g2ev")
                    nc.vector.tensor_copy(ev[:], acc[:])
                    nc.sync.dma_start(out=g2_o[mb], in_=ev[:])
                # G3[w, w'] partial: contract (o, d, h)
                acc3 = ps1.tile([W, W], f32, tag="g3acc", bufs=1)
                n = 0
                for d4 in range(D4):
                    for h in range(H):
                        nc.tensor.matmul(acc3[:], sv1r[:, d4, h, :], sv1r[:, d4, h, :],
                                         start=(n == 0), stop=(n == D4 * H - 1))
                        n += 1
                ev3 = ev1.tile([W, W], f32, tag="g3ev")
                nc.vector.tensor_copy(ev3[:], acc3[:])
                nc.sync.dma_start(out=g3_o[:], in_=ev3[:])

            # ---------------- phase 2: h-shard conv -> sv2; G1 --------------
            with tc.tile_pool(name="p2", bufs=1) as p2, \
                 tc.tile_pool(name="p2in", bufs=3) as p2i, \
                 tc.tile_pool(name="ps2", bufs=2, space="PSUM") as ps2, \
                 tc.tile_pool(name="ev2", bufs=2) as ev2:
                sv2 = p2.tile([128, F2], f32)
                sv2r = sv2.rearrange("p (d b w) -> p d b w", d=D, w=W)
                for dg in range(8):  # 8 pieces of 12 d
                    piece2 = p2i.tile([48, 12 * 288], f32, tag="st2piece")
                    off = dg * (12 * 288)
                    nc.sync.dma_start(out=piece2[:], in_=st2[:, off:off + 12 * 288])
                    for di in range(12):
                        ps = ps2.tile([128, 288], f32, tag="convps2")
                        nc.tensor.matmul(ps[:], w2_t[:], piece2[:, di * 288:(di + 1) * 288],
                                         start=True, stop=True)
                        nc.scalar.activation(
                            sv2[:, off + di * 288: off + (di + 1) * 288], ps[:],
                            mybir.ActivationFunctionType.Relu, bias=b_t[:])
                # G1[d, d'] partial: contract (o, h, w)
                acc1 = ps2.tile([D, D], f32, tag="g1acc", bufs=1)
                n = 0
                for h4 in range(H4):
                    for w in range(W):
                        nc.tensor.matmul(acc1[:], sv2r[:, :, h4, w], sv2r[:, :, h4, w],
                                         start=(n == 0), stop=(n == H4 * W - 1))
                        n += 1
                ev1t = ev2.tile([D, D], f32, tag="g1ev")
                nc.vector.tensor_copy(ev1t[:], acc1[:])
                nc.sync.dma_start(out=g1_o[:], in_=ev1t[:])

    nc.compile()
    return nc


def _get_bass():
    if "nc" not in _CACHE:
        _CACHE["nc"] = _build_bass()
    return _CACHE["nc"]


def _pack_inputs(state, conv_w, conv_b):
    """Host-side prep of per-core device inputs."""
    state = np.ascontiguousarray(state, dtype=np.float32)
    # block-diagonal conv weights [48, 128]: rows (j, i, kh), cols (j, o)
    w2 = conv_w[:, :, 0, :, 0].astype(np.float32)            # [O, I, KH]
    w2bd = np.zeros((48, 128), dtype=np.float32)
    for j in range(4):
        for i in range(I):
            for kh in range(KH):
                w2bd[12 * j + 3 * i + kh, 32 * j:32 * j + 32] = w2[:, i, kh]
    bias4 = np.tile(conv_b.astype(np.float32), 4).reshape(128, 1)
    bias4 = np.ascontiguousarray(bias4)

    in_maps = []
    for c in range(N_CORES):
        # st1 [48=(j,i,kh), d4, hb, 16, w]: state[i, 12c+4*d4+j, 16*hb+h'+kh, w]
        st1 = np.empty((4, I, KH, D4, HB, 16, W), dtype=np.float32)
        for j in range(4):
            for kh in range(KH):
                # [I, D4, H, W] slab for this (j, kh)
                slab = state[:, 12 * c + j: 12 * c + j + 4 * D4: 4, kh:kh + H, :]
                st1[j, :, kh] = slab.reshape(I, D4, HB, 16, W)
        st1 = st1.reshape(48, F1)
        # st2 [48=(j,i,kh), d, h4, w]: state[i, d, 48c+4*h4+j+kh, w]
        st2 = np.empty((4, I, KH, D, H4, W), dtype=np.float32)
        for j in range(4):
            for kh in range(KH):
                slab = state[:, :, 48 * c + j + kh: 48 * c + j + kh + 4 * H4: 4, :]
                st2[j, :, kh] = slab
        st2 = st2.reshape(48, F2)
        in_maps.append({
            "st1": np.ascontiguousarray(st1),
            "st2": np.ascontiguousarray(st2),
            "w2bd": w2bd,
            "bias4": bias4,
        })
    return in_maps


def run_device(state, conv_w, conv_b):
    """Run the SPMD NEFF; return (sv [O,D,H,W], G1 [D,D], G2 [H,H], G3 [W,W])."""
    from concourse import bass_utils
    nc = _get_bass()
    in_maps = _pack_inputs(state, conv_w, conv_b)
    res = bass_utils.run_bass_kernel_spmd(nc, in_maps, core_ids=list(range(N_CORES)))
    sv = np.empty((O, D, H, W), dtype=np.float32)
    G1 = np.zeros((D, D), dtype=np.float64)
    G2 = np.zeros((H, H), dtype=np.float64)
    G3 = np.zeros((W, W), dtype=np.float64)
    for c, r in enumerate(res.results):
        s1 = r["sv1"].reshape(4, O, D4, H, W)           # [(j, o), d4, h, w]
        sv[:, 12 * c: 12 * c + 12] = s1.transpose(1, 2, 0, 3, 4).reshape(O, 12, H, W)
        G1 += r["g1p"]
        G2 += r["g2p"].reshape(H, H)
        G3 += r["g3p"]
    return sv, G1.astype(np.float32), G2.astype(np.float32), G3.astype(np.float32)


# ---------------------------------------------------------------- host part
def _eigh(G):
    """eigh matching the reference's jax-CPU LAPACK (sign/degeneracy choices
    differ between LAPACK builds, and the final result is sign-sensitive)."""
    try:
        import jax
        import jax.numpy as jnp
        with jax.default_device(jax.devices("cpu")[0]):
            w, v = jnp.linalg.eigh(jnp.asarray(G, dtype=jnp.float32))
            return np.asarray(w), np.asarray(v)
    except Exception:
        return np.linalg.eigh(G)


def _top_left_vecs_from_gram(G, r):
    _, v = _eigh(G)
    return np.ascontiguousarray(v[:, ::-1][:, :r])


def _top_left_vecs(X, r):
    d, k = X.shape
    if d <= k:
        return _top_left_vecs_from_gram(X @ X.T, r)
    w, v = _eigh(X.T @ X)
    w = w[::-1][:r]
    v = np.ascontiguousarray(v[:, ::-1][:, :r])
    return (X @ v) / np.sqrt(np.maximum(w, np.float32(1e-12)))


def _unfold(t, n):
    return np.ascontiguousarray(np.moveaxis(t, n, 0)).reshape(t.shape[n], -1)


def _mode_dot_t(t, U, mode):
    return np.moveaxis(np.tensordot(t, U, axes=([mode], [0])), -1, mode)


def _project(t, factors, skip):
    order = sorted((m for m in range(t.ndim) if m != skip),
                   key=lambda m: factors[m].shape[1] / t.shape[m])
    y = t
    for m in order:
        y = _mode_dot_t(y, factors[m], m)
    return y


def _tucker_core(sv, G1, G2, G3):
    G0 = _unfold(sv, 0)
    G0 = G0 @ G0.T
    factors = [_top_left_vecs_from_gram(G, r)
               for G, r in zip((G0, G1, G2, G3), TUCKER_RANKS)]
    for _ in range(N_ITER):
        for n in range(4):
            y = _project(sv, factors, skip=n)
            factors[n] = _top_left_vecs(_unfold(y, n), TUCKER_RANKS[n])
    return _project(sv, factors, skip=-1)


def kernel(state, action, conv_w, conv_b, wa, ba, wq, bq):
    state = np.asarray(state, dtype=np.float32)
    action = np.asarray(action, dtype=np.float32)
    conv_w = np.asarray(conv_w, dtype=np.float32)
    conv_b = np.asarray(conv_b, dtype=np.float32)
    wa = np.asarray(wa, dtype=np.float32)
    ba = np.asarray(ba, dtype=np.float32)
    wq = np.asarray(wq, dtype=np.float32)
    bq = np.asarray(bq, dtype=np.float32)

    sv, G1, G2, G3 = run_device(state, conv_w, conv_b)
    core = _tucker_core(sv, G1, G2, G3)
    av = np.maximum(action @ wa.T + ba, 0.0).astype(np.float32)
    sav = np.maximum(core.reshape(-1) + av, 0.0).astype(np.float32)
    return (sav @ wq.T + bq).astype(np.float32)


# revision 9
# speedup vs baseline: 1.8118x; 1.8118x over previous
"""Trainium2 Bass kernel for nn_CriticNetwork_71442486001992.

Pipeline (per full call):
  device (8 NeuronCores, SPMD, one NEFF):
    - conv3d(state)+bias+relu in two packed layouts (d-sharded and h-sharded
      across cores), via a block-diagonal weight matmul (K=48)
    - mode-1/2/3 Gram partials of the conv output (the heavy FLOPs of the
      tucker HOSVD init), partial-summed per core
  host (numpy / LAPACK):
    - gram assembly + mode-0 gram, eigh-based HOSVD init, 5 HOOI sweeps
      (tiny tensors after the first contraction), final MLP head
Sharding: cores split mode-1 (d: 96 -> 12/core) for G2/G3 and mode-2
(h: 384 -> 48/core) for G1; gram partials are summed on host (no
collectives needed).
"""

import numpy as np

# ---------------------------------------------------------------- constants
N_CORES = 8
TUCKER_RANKS = (8, 2, 6, 2)
N_ITER = 5

O, I, KH = 32, 4, 3           # conv out-ch, in-ch, taps (along h)
D, H, W = 96, 384, 24         # sv spatial dims (d, h, w); input h = 386
DL = D // N_CORES             # 12 d's per core (mode-1 shard)
HL = H // N_CORES             # 48 h's per core (mode-2 shard)
D4, H4 = DL // 4, HL // 4     # 3, 12  (4-way partition packing)
HB = 24                       # h chunked into 24 blocks of 16 for conv pack1
F1 = D4 * H * W               # 27648 free elems of sv1  (d4, h, w)
F2 = D * H4 * W               # 27648 free elems of sv2  (d, h4, w)

_CACHE = {}


# ---------------------------------------------------------------- device part
def _build_bass(num_devices=N_CORES):
    import concourse.mybir as mybir
    from concourse import bacc
    from concourse.tile import TileContext

    f32 = mybir.dt.float32
    f32r = mybir.dt.float32r
    nc = bacc.Bacc("TRN2", target_bir_lowering=False, debug=False,
                   num_devices=num_devices)

    # inputs (per core)
    st1 = nc.dram_tensor("st1", [48, F1], f32r, kind="ExternalInput").ap()
    st2 = nc.dram_tensor("st2", [48, F2], f32r, kind="ExternalInput").ap()
    w2bd = nc.dram_tensor("w2bd", [48, 128], f32r, kind="ExternalInput").ap()
    bias4 = nc.dram_tensor("bias4", [128, 1], f32, kind="ExternalInput").ap()
    # outputs (per core)
    sv1_o = nc.dram_tensor("sv1", [128, F1], f32r, kind="ExternalOutput").ap()
    g2_o = nc.dram_tensor("g2p", [3, 128, H], f32, kind="ExternalOutput").ap()
    g1_o = nc.dram_tensor("g1p", [D, D], f32, kind="ExternalOutput").ap()
    g3_o = nc.dram_tensor("g3p", [W, W], f32, kind="ExternalOutput").ap()

    with TileContext(nc) as tc:
        with tc.tile_pool(name="consts", bufs=1) as cpool:
            w2_t = cpool.tile([48, 128], f32r)
            nc.sync.dma_start(out=w2_t[:], in_=w2bd[:])
            b_t = cpool.tile([128, 1], f32)
            nc.sync.dma_start(out=b_t[:], in_=bias4[:])

            # ---------------- phase 1: d-shard conv -> sv1; G2, G3 ----------
            with tc.tile_pool(name="p1", bufs=1) as p1, \
                 tc.tile_pool(name="p1in", bufs=3) as p1i, \
                 tc.tile_pool(name="ps1", bufs=2, space="PSUM") as ps1, \
                 tc.tile_pool(name="ev1", bufs=3) as ev1:
                sv1 = p1.tile([128, F1], f32r)
                sv1r = sv1.rearrange("p (a h w) -> p a h w", a=D4, w=W)
                # conv: 6 input pieces of (d4, hb-half); chunk = 16 h rows
                for d4 in range(D4):
                    for half in range(2):
                        piece = p1i.tile([48, 12 * 384], f32r, tag="st1piece")
                        off = d4 * (HB * 384) + half * (12 * 384)
                        nc.sync.dma_start(out=piece[:], in_=st1[:, off:off + 12 * 384])
                        for hb in range(12):
                            ps = ps1.tile([128, 384], f32, tag="convps")
                            nc.tensor.matmul(ps[:], w2_t[:], piece[:, hb * 384:(hb + 1) * 384],
                                             start=True, stop=True)
                            nc.scalar.activation(
                                sv1[:, off + hb * 384: off + (hb + 1) * 384], ps[:],
                                mybir.ActivationFunctionType.Relu, bias=b_t[:])
                # write sv1 back for the host
                nc.sync.dma_start(out=sv1_o[:], in_=sv1[:])
                # G2[h, h'] partial: contract (o, d, w); 3 psum row-blocks
                for mb in range(3):
                    acc = ps1.tile([128, H], f32, tag=f"g2acc{mb}", bufs=1)
                    n = 0
                    for d4 in range(D4):
                        for w in range(W):
                            nc.tensor.matmul(
                                acc[:],
                                sv1r[:, d4, mb * 128:(mb + 1) * 128, w],
                                sv1r[:, d4, :, w],
                                start=(n == 0), stop=(n == D4 * W - 1))
                            n += 1
                    ev = ev1.tile([128, H], f32, tag="g2ev")
                    nc.vector.tensor_copy(ev[:], acc[:])
                    nc.sync.dma_start(out=g2_o[mb], in_=ev[:])
                # G3[w, w'] partial: contract (o, d, h)
                acc3 = ps1.tile([W, W], f32, tag="g3acc", bufs=1)
                n = 0
                for d4 in range(D4):
                    for h in range(H):
                        nc.tensor.matmul(acc3[:], sv1r[:, d4, h, :], sv1r[:, d4, h, :],
                                         start=(n == 0), stop=(n == D4 * H - 1))
                        n += 1
                ev3 = ev1.tile([W, W], f32, tag="g3ev")
                nc.vector.tensor_copy(ev3[:], acc3[:])
                nc.sync.dma_start(out=g3_o[:], in_=ev3[:])

            # ---------------- phase 2: h-shard conv -> sv2; G1 --------------
            with tc.tile_pool(name="p2", bufs=1) as p2, \
                 tc.tile_pool(name="p2in", bufs=3) as p2i, \
                 tc.tile_pool(name="ps2", bufs=2, space="PSUM") as ps2, \
                 tc.tile_pool(name="ev2", bufs=2) as ev2:
                sv2 = p2.tile([128, F2], f32r)
                sv2r = sv2.rearrange("p (d b w) -> p d b w", d=D, w=W)
                for dg in range(8):  # 8 pieces of 12 d
                    piece2 = p2i.tile([48, 12 * 288], f32r, tag="st2piece")
                    off = dg * (12 * 288)
                    nc.sync.dma_start(out=piece2[:], in_=st2[:, off:off + 12 * 288])
                    for di in range(12):
                        ps = ps2.tile([128, 288], f32, tag="convps2")
                        nc.tensor.matmul(ps[:], w2_t[:], piece2[:, di * 288:(di + 1) * 288],
                                         start=True, stop=True)
                        nc.scalar.activation(
                            sv2[:, off + di * 288: off + (di + 1) * 288], ps[:],
                            mybir.ActivationFunctionType.Relu, bias=b_t[:])
                # G1[d, d'] partial: contract (o, h, w)
                acc1 = ps2.tile([D, D], f32, tag="g1acc", bufs=1)
                n = 0
                for h4 in range(H4):
                    for w in range(W):
                        nc.tensor.matmul(acc1[:], sv2r[:, :, h4, w], sv2r[:, :, h4, w],
                                         start=(n == 0), stop=(n == H4 * W - 1))
                        n += 1
                ev1t = ev2.tile([D, D], f32, tag="g1ev")
                nc.vector.tensor_copy(ev1t[:], acc1[:])
                nc.sync.dma_start(out=g1_o[:], in_=ev1t[:])

    nc.compile()
    return nc


def _get_bass():
    if "nc" not in _CACHE:
        _CACHE["nc"] = _build_bass()
    return _CACHE["nc"]


def _pack_inputs(state, conv_w, conv_b):
    """Host-side prep of per-core device inputs."""
    state = np.ascontiguousarray(state, dtype=np.float32)
    # block-diagonal conv weights [48, 128]: rows (j, i, kh), cols (j, o)
    w2 = conv_w[:, :, 0, :, 0].astype(np.float32)            # [O, I, KH]
    w2bd = np.zeros((48, 128), dtype=np.float32)
    for j in range(4):
        for i in range(I):
            for kh in range(KH):
                w2bd[12 * j + 3 * i + kh, 32 * j:32 * j + 32] = w2[:, i, kh]
    bias4 = np.tile(conv_b.astype(np.float32), 4).reshape(128, 1)
    bias4 = np.ascontiguousarray(bias4)

    in_maps = []
    for c in range(N_CORES):
        # st1 [48=(j,i,kh), d4, hb, 16, w]: state[i, 12c+4*d4+j, 16*hb+h'+kh, w]
        st1 = np.empty((4, I, KH, D4, HB, 16, W), dtype=np.float32)
        for j in range(4):
            for kh in range(KH):
                # [I, D4, H, W] slab for this (j, kh)
                slab = state[:, 12 * c + j: 12 * c + j + 4 * D4: 4, kh:kh + H, :]
                st1[j, :, kh] = slab.reshape(I, D4, HB, 16, W)
        st1 = st1.reshape(48, F1)
        # st2 [48=(j,i,kh), d, h4, w]: state[i, d, 48c+4*h4+j+kh, w]
        st2 = np.empty((4, I, KH, D, H4, W), dtype=np.float32)
        for j in range(4):
            for kh in range(KH):
                slab = state[:, :, 48 * c + j + kh: 48 * c + j + kh + 4 * H4: 4, :]
                st2[j, :, kh] = slab
        st2 = st2.reshape(48, F2)
        in_maps.append({
            "st1": np.ascontiguousarray(st1),
            "st2": np.ascontiguousarray(st2),
            "w2bd": w2bd,
            "bias4": bias4,
        })
    return in_maps


def run_device(state, conv_w, conv_b):
    """Run the SPMD NEFF; return (sv [O,D,H,W], G1 [D,D], G2 [H,H], G3 [W,W])."""
    from concourse import bass_utils
    nc = _get_bass()
    in_maps = _pack_inputs(state, conv_w, conv_b)
    res = bass_utils.run_bass_kernel_spmd(nc, in_maps, core_ids=list(range(N_CORES)))
    sv = np.empty((O, D, H, W), dtype=np.float32)
    G1 = np.zeros((D, D), dtype=np.float64)
    G2 = np.zeros((H, H), dtype=np.float64)
    G3 = np.zeros((W, W), dtype=np.float64)
    for c, r in enumerate(res.results):
        s1 = r["sv1"].reshape(4, O, D4, H, W)           # [(j, o), d4, h, w]
        sv[:, 12 * c: 12 * c + 12] = s1.transpose(1, 2, 0, 3, 4).reshape(O, 12, H, W)
        G1 += r["g1p"]
        G2 += r["g2p"].reshape(H, H)
        G3 += r["g3p"]
    return sv, G1.astype(np.float32), G2.astype(np.float32), G3.astype(np.float32)


# ---------------------------------------------------------------- host part
def _eigh(G):
    """eigh matching the reference's jax-CPU LAPACK (sign/degeneracy choices
    differ between LAPACK builds, and the final result is sign-sensitive)."""
    try:
        import jax
        import jax.numpy as jnp
        with jax.default_device(jax.devices("cpu")[0]):
            w, v = jnp.linalg.eigh(jnp.asarray(G, dtype=jnp.float32))
            return np.asarray(w), np.asarray(v)
    except Exception:
        return np.linalg.eigh(G)


def _top_left_vecs_from_gram(G, r):
    _, v = _eigh(G)
    return np.ascontiguousarray(v[:, ::-1][:, :r])


def _top_left_vecs(X, r):
    d, k = X.shape
    if d <= k:
        return _top_left_vecs_from_gram(X @ X.T, r)
    w, v = _eigh(X.T @ X)
    w = w[::-1][:r]
    v = np.ascontiguousarray(v[:, ::-1][:, :r])
    return (X @ v) / np.sqrt(np.maximum(w, np.float32(1e-12)))


def _unfold(t, n):
    return np.ascontiguousarray(np.moveaxis(t, n, 0)).reshape(t.shape[n], -1)


def _mode_dot_t(t, U, mode):
    return np.moveaxis(np.tensordot(t, U, axes=([mode], [0])), -1, mode)


def _project(t, factors, skip):
    order = sorted((m for m in range(t.ndim) if m != skip),
                   key=lambda m: factors[m].shape[1] / t.shape[m])
    y = t
    for m in order:
        y = _mode_dot_t(y, factors[m], m)
    return y


def _tucker_core(sv, G1, G2, G3):
    G0 = _unfold(sv, 0)
    G0 = G0 @ G0.T
    factors = [_top_left_vecs_from_gram(G, r)
               for G, r in zip((G0, G1, G2, G3), TUCKER_RANKS)]
    for _ in range(N_ITER):
        for n in range(4):
            y = _project(sv, factors, skip=n)
            factors[n] = _top_left_vecs(_unfold(y, n), TUCKER_RANKS[n])
    return _project(sv, factors, skip=-1)


def kernel(state, action, conv_w, conv_b, wa, ba, wq, bq):
    state = np.asarray(state, dtype=np.float32)
    action = np.asarray(action, dtype=np.float32)
    conv_w = np.asarray(conv_w, dtype=np.float32)
    conv_b = np.asarray(conv_b, dtype=np.float32)
    wa = np.asarray(wa, dtype=np.float32)
    ba = np.asarray(ba, dtype=np.float32)
    wq = np.asarray(wq, dtype=np.float32)
    bq = np.asarray(bq, dtype=np.float32)

    sv, G1, G2, G3 = run_device(state, conv_w, conv_b)
    core = _tucker_core(sv, G1, G2, G3)
    av = np.maximum(action @ wa.T + ba, 0.0).astype(np.float32)
    sav = np.maximum(core.reshape(-1) + av, 0.0).astype(np.float32)
    return (sav @ wq.T + bq).astype(np.float32)
